# revision 1
# baseline (speedup 1.0000x reference)
"""Trainium2 Bass kernel for nn_AttnModule_18141941858958 (gnn_message_passing).

Masked multi-head graph attention:
  q,k,v = per-head projections of node features; scores = q@k^T/sqrt(DH)
  masked by adjacency&node-mask; softmax; out = attn@v; concat heads;
  linear; ELU.

Strategy (single NeuronCore, all B=16 graphs on core 0 — see dispatch note):
  - Fold Wq@Wk^T/sqrt(DH) into a single [128,128] matrix M_h per head on the
    host: scores(q,k) = x_q . M_h . x_k, so no separate q/k projections and
    the scores matmul contracts over the full K=128.
  - Scores computed TRANSPOSED (sT[k,q]) so the probability matrix feeds the
    attn@V matmul directly as the moving operand (no transpose of p needed).
  - Mask applied multiplicatively after exp (0/1 bf16 matrix, host-prepped,
    transposed): pT = exp(sT) * allowT on the DVE (exact zeros, no PE cost).
  - Softmax denominator Z[q] and attn@V computed in ONE PE pass per k-tile
    via a combined [ones | v_kt] stationary tile (even 64-col slots ones,
    odd slots v): Z lands in PSUM partitions 0..63, attn@V in 64..127;
    normalization fuses into the PSUM->SBUF copy of attn-out.
  - Final linear computed transposed (yT[j,q]) in fp32r, then PE-transposed.
  - b_lin and bv folded on host; bq/bk terms vanish for zero biases
    (asserted; bk-side and constant terms are softmax-invariant).

Dispatch-path optimization: per-call cost through the axon PJRT tunnel is
dominated by per-operand and per-core dispatch overhead (~1ms per extra
input tensor at 8 cores, and rising with shard count), not by bytes.
Measured decomposition (single window, interleaved): empty-kernel floor
~2.3ms, this kernel ~2.5ms -> only ~0.2ms of the ~0.7ms/iter device time
(For_i repeat-loop slope) escapes overlap with the dispatch floor.  ALL
runtime inputs are packed into a single flat fp32 blob per core
(xT | mask | Mh | Wv | Wl | blin); the identity matrix is an inline NEFF
constant; the partition-id parameter is disabled; and the whole batch runs
on ONE core (measured per-call wall: 1c=1.8ms, 2c=3.3ms, 4c=4.2ms,
8c=4.9ms, vs 11.5ms for the 8-core 10-operand baseline on the same box).
Net: 2 PJRT operands (blob in, y out) instead of 10.  Every blob section is
stored row-major CONTIGUOUS so each load is a single dense DMA run (strided
DRAM access patterns cost ~128 descriptors per DMA per call on this path),
all through hardware DGE with on-chip DVE casts — no SWDGE in the NEFF.
"""

import sys

sys.path.insert(0, "/opt/trn_rl_repo")

import numpy as np

B, N, DIN, H, DH, DO, DLIN = 16, 512, 128, 8, 64, 64, 128
NCORES = 1
BL = B // NCORES  # graphs per core
NT = N // 128  # 128-node tiles per graph

# flat blob layout (per core, [TOTAL] fp32): every section is a [128, X]
# tile stored row-major CONTIGUOUS, so each DMA is one dense run (strided
# DRAM access patterns cost ~128 descriptors per DMA on this path).
GW = 128 * N  # words per xT graph section
AW = 128 * NT * N  # words per allow graph section (one contiguous block)
XB = 0  # xT: BL x GW
AB = XB + BL * GW  # allow: BL x AW
MB = AB + BL * AW  # Mh: 128 x H*DIN
VB = MB + 128 * H * DIN  # Wv: 128 x H*DO
LB = VB + 128 * H * DO  # Wl: 128 x 4*DLIN
BB = LB + 128 * 4 * DLIN  # blin: 128 x 1
TOTAL = BB + 128

_CACHE = {}


def _build_nc(repeat=1):
    import concourse.tile as tile
    from concourse import bacc, mybir
    from contextlib import ExitStack

    F32 = mybir.dt.float32
    F32R = mybir.dt.float32r
    BF16 = mybir.dt.bfloat16
    EXP = mybir.ActivationFunctionType.Exp
    RECIP = mybir.ActivationFunctionType.Reciprocal
    RELU = mybir.ActivationFunctionType.Relu
    ALU = mybir.AluOpType

    nc = bacc.Bacc(
        "TRN2",
        target_bir_lowering=False,
        debug=False,
        enable_asserts=False,
        num_devices=NCORES,
        enable_partition_id=False,
    )

    blob_d = nc.dram_tensor("blob", [TOTAL], F32R, kind="ExternalInput").ap()

    def sec(off, cols):  # contiguous [128, cols] view at word offset
        return blob_d[off : off + 128 * cols].rearrange("(p x) -> p x", p=128)
    id_d = nc.inline_tensor(np.eye(128, dtype=np.float32), name="ident_c").ap()
    y_d = nc.dram_tensor("y", [BL, N, DLIN], F32, kind="ExternalOutput").ap()

    with tile.TileContext(nc) as tc:
        ctx = ExitStack()
        consts = ctx.enter_context(tc.tile_pool(name="consts", bufs=1))
        wpool = ctx.enter_context(tc.tile_pool(name="weights", bufs=1))
        xpool = ctx.enter_context(tc.tile_pool(name="x", bufs=2))
        apool = ctx.enter_context(tc.tile_pool(name="allow", bufs=2))
        gpool = ctx.enter_context(tc.tile_pool(name="g", bufs=4))
        zvpool = ctx.enter_context(tc.tile_pool(name="zvp", bufs=1))
        ppool = ctx.enter_context(tc.tile_pool(name="p", bufs=3))
        rpool = ctx.enter_context(tc.tile_pool(name="rz", bufs=4))
        spool = ctx.enter_context(tc.tile_pool(name="stack", bufs=8))
        ypool = ctx.enter_context(tc.tile_pool(name="yy", bufs=2))
        ps_s = ctx.enter_context(tc.tile_pool(name="ps_s", bufs=3, space="PSUM"))
        ps_o = ctx.enter_context(tc.tile_pool(name="ps_o", bufs=2, space="PSUM"))

        # constants
        ident = consts.tile([128, 128], F32, name="ident")
        nc.sync.dma_start(ident[:], id_d[:])
        blin_f = consts.tile([128, 1], F32R, name="blin_f")
        nc.sync.dma_start(blin_f[:], sec(BB, 1))
        blin = consts.tile([128, 1], F32, name="blin")
        nc.vector.tensor_copy(blin[:], blin_f[:])
        nblin = consts.tile([128, 1], F32, name="nblin")
        nc.scalar.mul(nblin[:], blin[:], -1.0)

        # weights (replicated across cores), carved from the blob
        Mh = wpool.tile([128, H * DIN], F32R, name="Mh")
        nc.sync.dma_start(Mh[:], sec(MB, H * DIN))
        Wv_f = wpool.tile([128, H * DO], F32R, name="Wv_f")
        nc.sync.dma_start(Wv_f[:], sec(VB, H * DO))
        Wv = wpool.tile([128, H * DO], BF16, name="Wv")
        nc.vector.tensor_copy(Wv[:], Wv_f[:])
        Wl = wpool.tile([128, 4 * DLIN], F32R, name="Wl")
        nc.sync.dma_start(Wl[:], sec(LB, 4 * DLIN))

        # persistent [ones | v] stationary tiles: ones (even 64-col slots) are
        # memset ONCE here; per-unit stageA only rewrites the odd v slots, so
        # the ones survive tile reuse (8 tiles cover the pipeline depth).
        NZV = 8
        zv_tiles = []
        for j in range(NZV):
            z = zvpool.tile([128, 2 * NT * DO], BF16, name=f"zvt{j}")
            nc.vector.memset(z[:], 1.0)
            zv_tiles.append(z)

        rep_ctx = tc.For_i(0, repeat, 1) if repeat > 1 else None
        if rep_ctx is not None:
            rep_ctx.__enter__()

        units = [(b, h) for b in range(BL) for h in range(H)]
        st = {}
        graphs = {}

        def load_graph(b):
            xs = sec(XB + b * GW, N)
            xT = xpool.tile([128, N], F32R, name=f"xT{b}", tag="xT")
            nc.sync.dma_start(xT[:], xs)
            xbf = xpool.tile([128, N], BF16, name=f"xbf{b}", tag="xbf")
            nc.vector.tensor_copy(xbf[:], xT[:])
            af = apool.tile([128, NT * N], F32R, name=f"alwf{b}", tag="alwf")
            nc.scalar.dma_start(af[:], sec(AB + b * AW, NT * N))
            alw = apool.tile([128, NT * N], BF16, name=f"alw{b}", tag="alw")
            nc.vector.tensor_copy(alw[:], af[:])
            graphs[b] = dict(xT=xT, xbf=xbf, alw=alw, stacks=[])

        def stageA(u):
            b, h = u
            if h == 0:
                load_graph(b)
            G = graphs[b]
            xT, xbf = G["xT"], G["xbf"]
            g_ps = ps_s.tile([128, 2 * N], F32, name=f"gps{b}_{h}", tag="sps")
            nc.tensor.matmul(
                g_ps[:, 0:N], Mh[:, h * 128 : (h + 1) * 128], xT[:],
                start=True, stop=True,
            )
            gT = gpool.tile([128, N], F32R, name=f"gT{b}_{h}", tag="gT")
            nc.vector.tensor_copy(gT[:], g_ps[:, 0:N])
            v_ps = ps_o.tile([128, NT * DO], F32, name=f"vps{b}_{h}", tag="ops")
            for t in range(NT):
                nc.tensor.matmul(
                    v_ps[:, t * DO : (t + 1) * DO],
                    xbf[:, t * 128 : (t + 1) * 128],
                    Wv[:, h * DO : (h + 1) * DO],
                    start=True, stop=True,
                )
            # combined [ones | v_kt] stationary: even 64-col slots are all-ones
            # (softmax denominator Z), odd slots are v for tile kt (attn@V).
            zv = zv_tiles[(b * H + h) % NZV]
            nc.vector.tensor_copy(
                zv[:].rearrange("p (t two c) -> p t two c", two=2, c=DO)[:, :, 1, :],
                v_ps[:].rearrange("p (t c) -> p t c", c=DO),
            )
            st[u] = dict(gT=gT, zv=zv)

        def stageB(u):
            b, h = u
            G = graphs[b]
            xT = G["xT"]
            gT = st[u]["gT"]
            pT = ppool.tile([128, NT * N], BF16, name=f"pT{b}_{h}", tag="pT")
            pe = ppool.tile([128, NT * N], BF16, name=f"pe{b}_{h}", tag="pe")
            for half in range(2):
                s_ps = ps_s.tile(
                    [128, 2 * N], F32, name=f"sps{b}_{h}_{half}", tag="sps"
                )
                for k2 in range(2):
                    kt = 2 * half + k2
                    nc.tensor.matmul(
                        s_ps[:, k2 * N : (k2 + 1) * N],
                        xT[:, kt * 128 : (kt + 1) * 128],
                        gT[:],
                        start=True, stop=True,
                    )
                sl = slice(half * 2 * N, (half + 1) * 2 * N)
                nc.scalar.activation(pe[:, sl], s_ps[:], EXP)
            nc.vector.tensor_mul(pT[:], pe[:], G["alw"][:])
            st[u]["pT"] = pT

        def stageC(u):
            b, h = u
            G = graphs[b]
            pT, zv = st[u]["pT"], st[u]["zv"]
            if h % 2 == 0:
                stk = spool.tile([128, N], F32R, name=f"stk{b}_{h//2}", tag="stk")
                G["stacks"].append(stk)
            stk = G["stacks"][-1]
            o_ps = ps_o.tile([128, N], F32, name=f"ops{b}_{h}", tag="ops")
            for kt in range(NT):
                nc.tensor.matmul(
                    o_ps[:, :],
                    zv[:, 2 * kt * DO : (2 * kt + 2) * DO],
                    pT[:, kt * N : (kt + 1) * N],
                    start=(kt == 0), stop=(kt == NT - 1),
                )
            rzb = rpool.tile([DO, N], F32, name=f"rzb{b}_{h}", tag="rzb")
            nc.vector.reciprocal_approx_fast(rzb[:], o_ps[0:DO, :])
            nc.vector.tensor_mul(
                stk[(h % 2) * DO : (h % 2 + 1) * DO, :],
                o_ps[64:128, :],
                rzb[:],
            )
            if h == H - 1:
                tail_y(b)

        def tail_y(b):
            G = graphs[b]
            yt_ps = ps_s.tile([128, 2 * N], F32, name=f"ytps{b}", tag="sps")
            for t in range(4):
                nc.tensor.matmul(
                    yt_ps[:, 0:N],
                    Wl[:, t * DLIN : (t + 1) * DLIN],
                    G["stacks"][t][:],
                    start=(t == 0), stop=(t == 3),
                )
            rn_sb = ypool.tile([128, N], F32, name=f"rn{b}", tag="rn")
            nc.scalar.activation(rn_sb[:], yt_ps[:, 0:N], RELU, bias=nblin[:], scale=-1.0)
            e_sb = ypool.tile([128, N], F32, name=f"e{b}", tag="e")
            nc.scalar.activation(e_sb[:], rn_sb[:], EXP, scale=-1.0)
            r_sb = ypool.tile([128, N], F32, name=f"r{b}", tag="r")
            nc.scalar.activation(r_sb[:], yt_ps[:, 0:N], RELU, bias=blin[:])
            yf = ypool.tile([128, N], F32, name=f"yf{b}", tag="yf")
            nc.vector.scalar_tensor_tensor(
                yf[:], r_sb[:], -1.0, e_sb[:], op0=ALU.add, op1=ALU.add
            )
            for qt in range(NT):
                tr_ps = ps_o.tile([128, 128], F32, name=f"tr{b}_{qt}", tag="ops")
                nc.tensor.transpose(
                    tr_ps[:], yf[:, qt * 128 : (qt + 1) * 128], ident[:]
                )
                y_sb = ypool.tile([128, 128], F32, name=f"ysb{b}_{qt}", tag="ysb")
                nc.vector.tensor_copy(y_sb[:], tr_ps[:])
                nc.sync.dma_start(y_d[b, qt * 128 : (qt + 1) * 128, :], y_sb[:])

        NU = len(units)
        for i in range(NU + 2):
            if i < NU:
                stageA(units[i])
            if 1 <= i <= NU:
                stageB(units[i - 1])
            if 2 <= i <= NU + 1:
                stageC(units[i - 2])

        if rep_ctx is not None:
            rep_ctx.__exit__(None, None, None)
        ctx.close()

    nc.compile()
    return nc


def _get_nc(repeat=1):
    key = f"nc{repeat}"
    if key not in _CACHE:
        _CACHE[key] = _build_nc(repeat)
    return _CACHE[key]


def _host_prep(node_features, masks, adj, Wq, Wk, Wv, bq, bk, bv, W_lin, b_lin):
    nf = np.asarray(node_features, np.float32)
    masks = np.asarray(masks)
    adj = np.asarray(adj)
    Wq = np.asarray(Wq, np.float32)
    Wk = np.asarray(Wk, np.float32)
    Wv_ = np.asarray(Wv, np.float32)
    bq = np.asarray(bq, np.float32)
    bv_ = np.asarray(bv, np.float32)
    W_lin = np.asarray(W_lin, np.float32)
    b_lin = np.asarray(b_lin, np.float32)

    # bq contributes a per-k additive score term x_k.(Wk@bq); zero in this
    # problem's setup_inputs.  (bk-side and constant terms are softmax-
    # invariant and drop exactly.)
    assert np.abs(bq).max() == 0.0, "nonzero bq not supported by fast path"

    xT = np.ascontiguousarray(nf.transpose(0, 2, 1))  # [B, DIN, N]
    allow = (adj != 0) & (masks != 0)[:, None, :]  # [B, q, k]
    allowT = allow.transpose(0, 2, 1)  # [B, k, q]
    allowT = (
        allowT
        .reshape(B, NT, 128, N)
        .transpose(0, 2, 1, 3)
        .reshape(B, 128, NT * N)
        .astype(np.float32)
    )
    scale = 1.0 / np.sqrt(DH)
    M = np.einsum("hde,hfe->hdf", Wq, Wk).astype(np.float32) * scale  # [H,DIN,DIN]
    Mh = np.ascontiguousarray(M.transpose(1, 0, 2).reshape(DIN, H * DIN))
    Wv_p = np.ascontiguousarray(Wv_.transpose(1, 0, 2).reshape(DIN, H * DO))
    Wl_p = np.ascontiguousarray(
        W_lin.reshape(4, 128, DLIN).transpose(1, 0, 2).reshape(128, 4 * DLIN)
    )
    blin_eff = (b_lin + bv_.reshape(H * DO) @ W_lin).reshape(DLIN, 1)
    return xT, allowT, Mh, Wv_p, Wl_p, blin_eff


def make_in_maps(**inputs):
    xT, allowT, Mh, Wv_p, Wl_p, blin_eff = _host_prep(**inputs)
    in_maps = []
    for c in range(NCORES):
        blob = np.empty(TOTAL, np.float32)
        for j in range(BL):
            b = c * BL + j
            blob[XB + j * GW : XB + (j + 1) * GW] = xT[b].ravel()
            blob[AB + j * AW : AB + (j + 1) * AW] = allowT[b].ravel()
        blob[MB : MB + 128 * H * DIN] = Mh.ravel()
        blob[VB : VB + 128 * H * DO] = Wv_p.ravel()
        blob[LB : LB + 128 * 4 * DLIN] = Wl_p.ravel()
        blob[BB : BB + 128] = blin_eff.astype(np.float32).ravel()
        in_maps.append({"blob": blob})
    return in_maps


def kernel(**inputs):
    from concourse import bass_utils

    nc = _get_nc()
    in_maps = make_in_maps(**inputs)
    res = bass_utils.run_bass_kernel_spmd(nc, in_maps, core_ids=list(range(NCORES)))
    y = np.concatenate([res.results[c]["y"] for c in range(NCORES)], axis=0)
    return np.ascontiguousarray(y.astype(np.float32))



# revision 7
# speedup vs baseline: 14.6936x; 14.6936x over previous
"""Trainium2 Bass kernel for nn_AttnModule_18141941858958 (gnn_message_passing).

Masked multi-head graph attention:
  q,k,v = per-head projections of node features; scores = q@k^T/sqrt(DH)
  masked by adjacency&node-mask; softmax; out = attn@v; concat heads;
  linear; ELU.

Strategy (single NeuronCore, all B=16 graphs on core 0 — see dispatch note):
  - Fold Wq@Wk^T/sqrt(DH) into a single [128,128] matrix M_h per head on the
    host: scores(q,k) = x_q . M_h . x_k, so no separate q/k projections and
    the scores matmul contracts over the full K=128.
  - Scores computed TRANSPOSED (sT[k,q]) so the probability matrix feeds the
    attn@V matmul directly as the moving operand (no transpose of p needed).
  - Mask applied multiplicatively after exp (0/1 bf16 matrix, host-prepped,
    transposed): pT = exp(sT) * allowT on the DVE (exact zeros, no PE cost).
  - Softmax denominator Z[q] and attn@V computed in ONE PE pass per k-tile
    via a combined [ones | v_kt] stationary tile (even 64-col slots ones,
    odd slots v): Z lands in PSUM partitions 0..63, attn@V in 64..127;
    normalization fuses into the PSUM->SBUF copy of attn-out.
  - Final linear computed transposed (yT[j,q]) in fp32r, then PE-transposed.
  - b_lin and bv folded on host; bq/bk terms vanish for zero biases
    (asserted; bk-side and constant terms are softmax-invariant).

Dispatch-path optimization: per-call cost through the axon PJRT tunnel is
dominated by per-operand and per-core dispatch overhead (~1ms per extra
input tensor at 8 cores, and rising with shard count), not by bytes.
Measured decomposition (single window, interleaved): empty-kernel floor
~2.3ms, this kernel ~2.5ms -> only ~0.2ms of the ~0.7ms/iter device time
(For_i repeat-loop slope) escapes overlap with the dispatch floor.  ALL
runtime inputs are packed into a single flat fp32 blob per core
(xT | mask | Mh | Wv | Wl | blin); the identity matrix is an inline NEFF
constant; the partition-id parameter is disabled; and the whole batch runs
on ONE core (measured per-call wall: 1c=1.8ms, 2c=3.3ms, 4c=4.2ms,
8c=4.9ms, vs 11.5ms for the 8-core 10-operand baseline on the same box).
Net: 2 PJRT operands (blob in, y out) instead of 10.  Every blob section is
stored row-major CONTIGUOUS so each load is a single dense DMA run (strided
DRAM access patterns cost ~128 descriptors per DMA per call on this path),
all through hardware DGE with on-chip DVE casts — no SWDGE in the NEFF.
"""

import sys

sys.path.insert(0, "/opt/trn_rl_repo")

import numpy as np

B, N, DIN, H, DH, DO, DLIN = 16, 512, 128, 8, 64, 64, 128
NCORES = 8
BL = B // NCORES  # graphs per core
NT = N // 128  # 128-node tiles per graph

# flat blob layout (per core, [TOTAL] fp32): every section is a [128, X]
# tile stored row-major CONTIGUOUS, so each DMA is one dense run (strided
# DRAM access patterns cost ~128 descriptors per DMA on this path).
# The allow mask is packed as u8 BYTES inside the fp32 blob (bitcast view
# on the device side): 4x less HBM traffic than fp32, cast u8->bf16 on DVE.
GW = 128 * N  # words per xT graph section
AWW = 128 * NT * N // 4  # fp32 WORDS per u8-packed allow graph section
XB = 0  # xT: BL x GW
AB = XB + BL * GW  # allow (u8 packed): BL x AWW words
MB = AB + BL * AWW  # Mh: 128 x H*DIN
VB = MB + 128 * H * DIN  # Wv: 128 x H*DO
LB = VB + 128 * H * DO  # Wl: 128 x 4*DLIN
BB = LB + 128 * 4 * DLIN  # blin: 128 x 1
TOTAL = BB + 128

_CACHE = {}


def _build_nc(repeat=1):
    import concourse.tile as tile
    from concourse import bacc, mybir
    from contextlib import ExitStack

    F32 = mybir.dt.float32
    F32R = mybir.dt.float32r
    BF16 = mybir.dt.bfloat16
    U8 = mybir.dt.uint8
    EXP = mybir.ActivationFunctionType.Exp
    RECIP = mybir.ActivationFunctionType.Reciprocal
    RELU = mybir.ActivationFunctionType.Relu
    ALU = mybir.AluOpType

    nc = bacc.Bacc(
        "TRN2",
        target_bir_lowering=False,
        debug=False,
        enable_asserts=False,
        num_devices=NCORES,
        enable_partition_id=False,
    )

    blob_d = nc.dram_tensor("blob", [TOTAL], F32R, kind="ExternalInput").ap()

    def sec(off, cols):  # contiguous [128, cols] view at word offset
        return blob_d[off : off + 128 * cols].rearrange("(p x) -> p x", p=128)

    blob_u8 = blob_d.bitcast(mybir.dt.uint8)

    def secu8(off_words, cols):  # contiguous u8 [128, cols] view
        return blob_u8[off_words * 4 : off_words * 4 + 128 * cols].rearrange(
            "(p x) -> p x", p=128
        )
    id_d = nc.inline_tensor(np.eye(128, dtype=np.float32), name="ident_c").ap()
    y_d = nc.dram_tensor("y", [BL, N, DLIN], F32, kind="ExternalOutput").ap()

    with tile.TileContext(nc) as tc:
        ctx = ExitStack()
        consts = ctx.enter_context(tc.tile_pool(name="consts", bufs=1))
        wpool = ctx.enter_context(tc.tile_pool(name="weights", bufs=1))
        xpool = ctx.enter_context(tc.tile_pool(name="x", bufs=2))
        apool = ctx.enter_context(tc.tile_pool(name="allow", bufs=2))
        gpool = ctx.enter_context(tc.tile_pool(name="g", bufs=4))
        zvpool = ctx.enter_context(tc.tile_pool(name="zvp", bufs=1))
        ppool = ctx.enter_context(tc.tile_pool(name="p", bufs=3))
        rpool = ctx.enter_context(tc.tile_pool(name="rz", bufs=4))
        spool = ctx.enter_context(tc.tile_pool(name="stack", bufs=8))
        ypool = ctx.enter_context(tc.tile_pool(name="yy", bufs=2))
        ps_s = ctx.enter_context(tc.tile_pool(name="ps_s", bufs=3, space="PSUM"))
        ps_o = ctx.enter_context(tc.tile_pool(name="ps_o", bufs=2, space="PSUM"))

        # constants
        ident = consts.tile([128, 128], F32, name="ident")
        nc.sync.dma_start(ident[:], id_d[:])
        blin_f = consts.tile([128, 1], F32R, name="blin_f")
        nc.sync.dma_start(blin_f[:], sec(BB, 1))
        blin = consts.tile([128, 1], F32, name="blin")
        nc.vector.tensor_copy(blin[:], blin_f[:])
        nblin = consts.tile([128, 1], F32, name="nblin")
        nc.scalar.mul(nblin[:], blin[:], -1.0)

        # weights (replicated across cores), carved from the blob
        Mh = wpool.tile([128, H * DIN], F32R, name="Mh")
        nc.sync.dma_start(Mh[:], sec(MB, H * DIN))
        Wv_f = wpool.tile([128, H * DO], F32R, name="Wv_f")
        nc.sync.dma_start(Wv_f[:], sec(VB, H * DO))
        Wv = wpool.tile([128, H * DO], BF16, name="Wv")
        nc.vector.tensor_copy(Wv[:], Wv_f[:])
        Wl = wpool.tile([128, 4 * DLIN], F32R, name="Wl")
        nc.sync.dma_start(Wl[:], sec(LB, 4 * DLIN))

        # persistent [ones | v] stationary tiles: ones (even 64-col slots) are
        # memset ONCE here; per-unit stageA only rewrites the odd v slots, so
        # the ones survive tile reuse (8 tiles cover the pipeline depth).
        NZV = 8
        zv_tiles = []
        for j in range(NZV):
            z = zvpool.tile([128, 2 * NT * DO], BF16, name=f"zvt{j}")
            nc.vector.memset(z[:], 1.0)
            zv_tiles.append(z)

        rep_ctx = tc.For_i(0, repeat, 1) if repeat > 1 else None
        if rep_ctx is not None:
            rep_ctx.__enter__()

        units = [(b, h) for b in range(BL) for h in range(H)]
        st = {}
        graphs = {}

        def load_graph(b):
            xs = sec(XB + b * GW, N)
            xT = xpool.tile([128, N], F32R, name=f"xT{b}", tag="xT")
            nc.sync.dma_start(xT[:], xs)
            xbf = xpool.tile([128, N], BF16, name=f"xbf{b}", tag="xbf")
            nc.vector.tensor_copy(xbf[:], xT[:])
            af = apool.tile([128, NT * N], U8, name=f"alwu{b}", tag="alwu")
            nc.scalar.dma_start(af[:], secu8(AB + b * AWW, NT * N))
            alw = apool.tile([128, NT * N], BF16, name=f"alw{b}", tag="alw")
            nc.vector.tensor_copy(alw[:], af[:])
            graphs[b] = dict(xT=xT, xbf=xbf, alw=alw, stacks=[])

        def stageA(u):
            b, h = u
            if h == 0:
                load_graph(b)
            G = graphs[b]
            xT, xbf = G["xT"], G["xbf"]
            g_ps = ps_s.tile([128, 2 * N], F32, name=f"gps{b}_{h}", tag="sps")
            nc.tensor.matmul(
                g_ps[:, 0:N], Mh[:, h * 128 : (h + 1) * 128], xT[:],
                start=True, stop=True,
            )
            gT = gpool.tile([128, N], F32R, name=f"gT{b}_{h}", tag="gT")
            nc.vector.tensor_copy(gT[:], g_ps[:, 0:N])
            v_ps = ps_o.tile([128, NT * DO], F32, name=f"vps{b}_{h}", tag="ops")
            for t in range(NT):
                nc.tensor.matmul(
                    v_ps[:, t * DO : (t + 1) * DO],
                    xbf[:, t * 128 : (t + 1) * 128],
                    Wv[:, h * DO : (h + 1) * DO],
                    start=True, stop=True,
                )
            # combined [ones | v_kt] stationary: even 64-col slots are all-ones
            # (softmax denominator Z), odd slots are v for tile kt (attn@V).
            zv = zv_tiles[(b * H + h) % NZV]
            nc.vector.tensor_copy(
                zv[:].rearrange("p (t two c) -> p t two c", two=2, c=DO)[:, :, 1, :],
                v_ps[:].rearrange("p (t c) -> p t c", c=DO),
            )
            st[u] = dict(gT=gT, zv=zv)

        def stageB(u):
            b, h = u
            G = graphs[b]
            xT = G["xT"]
            gT = st[u]["gT"]
            pT = ppool.tile([128, NT * N], BF16, name=f"pT{b}_{h}", tag="pT")
            pe = ppool.tile([128, NT * N], BF16, name=f"pe{b}_{h}", tag="pe")
            for half in range(2):
                s_ps = ps_s.tile(
                    [128, 2 * N], F32, name=f"sps{b}_{h}_{half}", tag="sps"
                )
                for k2 in range(2):
                    kt = 2 * half + k2
                    nc.tensor.matmul(
                        s_ps[:, k2 * N : (k2 + 1) * N],
                        xT[:, kt * 128 : (kt + 1) * 128],
                        gT[:],
                        start=True, stop=True,
                    )
                sl = slice(half * 2 * N, (half + 1) * 2 * N)
                nc.scalar.activation(pe[:, sl], s_ps[:], EXP)
            nc.vector.tensor_mul(pT[:], pe[:], G["alw"][:])
            st[u]["pT"] = pT

        def stageC(u):
            b, h = u
            G = graphs[b]
            pT, zv = st[u]["pT"], st[u]["zv"]
            if h % 2 == 0:
                stk = spool.tile([128, N], F32R, name=f"stk{b}_{h//2}", tag="stk")
                G["stacks"].append(stk)
            stk = G["stacks"][-1]
            o_ps = ps_o.tile([128, N], F32, name=f"ops{b}_{h}", tag="ops")
            for kt in range(NT):
                nc.tensor.matmul(
                    o_ps[:, :],
                    zv[:, 2 * kt * DO : (2 * kt + 2) * DO],
                    pT[:, kt * N : (kt + 1) * N],
                    start=(kt == 0), stop=(kt == NT - 1),
                )
            rzb = rpool.tile([DO, N], F32, name=f"rzb{b}_{h}", tag="rzb")
            nc.vector.reciprocal_approx_fast(rzb[:], o_ps[0:DO, :])
            nc.vector.tensor_mul(
                stk[(h % 2) * DO : (h % 2 + 1) * DO, :],
                o_ps[64:128, :],
                rzb[:],
            )
            if h == H - 1:
                tail_y(b)

        def tail_y(b):
            G = graphs[b]
            yt_ps = ps_s.tile([128, 2 * N], F32, name=f"ytps{b}", tag="sps")
            for t in range(4):
                nc.tensor.matmul(
                    yt_ps[:, 0:N],
                    Wl[:, t * DLIN : (t + 1) * DLIN],
                    G["stacks"][t][:],
                    start=(t == 0), stop=(t == 3),
                )
            rn_sb = ypool.tile([128, N], F32, name=f"rn{b}", tag="rn")
            nc.scalar.activation(rn_sb[:], yt_ps[:, 0:N], RELU, bias=nblin[:], scale=-1.0)
            e_sb = ypool.tile([128, N], F32, name=f"e{b}", tag="e")
            nc.scalar.activation(e_sb[:], rn_sb[:], EXP, scale=-1.0)
            r_sb = ypool.tile([128, N], F32, name=f"r{b}", tag="r")
            nc.scalar.activation(r_sb[:], yt_ps[:, 0:N], RELU, bias=blin[:])
            yf = ypool.tile([128, N], F32, name=f"yf{b}", tag="yf")
            nc.vector.scalar_tensor_tensor(
                yf[:], r_sb[:], -1.0, e_sb[:], op0=ALU.add, op1=ALU.add
            )
            for qt in range(NT):
                tr_ps = ps_o.tile([128, 128], F32, name=f"tr{b}_{qt}", tag="ops")
                nc.tensor.transpose(
                    tr_ps[:], yf[:, qt * 128 : (qt + 1) * 128], ident[:]
                )
                y_sb = ypool.tile([128, 128], F32, name=f"ysb{b}_{qt}", tag="ysb")
                nc.vector.tensor_copy(y_sb[:], tr_ps[:])
                nc.sync.dma_start(y_d[b, qt * 128 : (qt + 1) * 128, :], y_sb[:])

        NU = len(units)
        for i in range(NU + 2):
            if i < NU:
                stageA(units[i])
            if 1 <= i <= NU:
                stageB(units[i - 1])
            if 2 <= i <= NU + 1:
                stageC(units[i - 2])

        if rep_ctx is not None:
            rep_ctx.__exit__(None, None, None)
        ctx.close()

    nc.compile()
    return nc


def _get_nc(repeat=1):
    key = f"nc{repeat}"
    if key not in _CACHE:
        _CACHE[key] = _build_nc(repeat)
    return _CACHE[key]


def _host_prep(node_features, masks, adj, Wq, Wk, Wv, bq, bk, bv, W_lin, b_lin):
    nf = np.asarray(node_features, np.float32)
    masks = np.asarray(masks)
    adj = np.asarray(adj)
    Wq = np.asarray(Wq, np.float32)
    Wk = np.asarray(Wk, np.float32)
    Wv_ = np.asarray(Wv, np.float32)
    bq = np.asarray(bq, np.float32)
    bv_ = np.asarray(bv, np.float32)
    W_lin = np.asarray(W_lin, np.float32)
    b_lin = np.asarray(b_lin, np.float32)

    # bq contributes a per-k additive score term x_k.(Wk@bq); zero in this
    # problem's setup_inputs.  (bk-side and constant terms are softmax-
    # invariant and drop exactly.)
    assert np.abs(bq).max() == 0.0, "nonzero bq not supported by fast path"

    xT = np.ascontiguousarray(nf.transpose(0, 2, 1))  # [B, DIN, N]
    allow = (adj != 0) & (masks != 0)[:, None, :]  # [B, q, k]
    allowT = allow.transpose(0, 2, 1)  # [B, k, q]
    allowT = (
        allowT
        .reshape(B, NT, 128, N)
        .transpose(0, 2, 1, 3)
        .reshape(B, 128, NT * N)
        .astype(np.uint8)
    )
    scale = 1.0 / np.sqrt(DH)
    M = np.einsum("hde,hfe->hdf", Wq, Wk).astype(np.float32) * scale  # [H,DIN,DIN]
    Mh = np.ascontiguousarray(M.transpose(1, 0, 2).reshape(DIN, H * DIN))
    Wv_p = np.ascontiguousarray(Wv_.transpose(1, 0, 2).reshape(DIN, H * DO))
    Wl_p = np.ascontiguousarray(
        W_lin.reshape(4, 128, DLIN).transpose(1, 0, 2).reshape(128, 4 * DLIN)
    )
    blin_eff = (b_lin + bv_.reshape(H * DO) @ W_lin).reshape(DLIN, 1)
    return xT, allowT, Mh, Wv_p, Wl_p, blin_eff


def make_in_maps(**inputs):
    xT, allowT, Mh, Wv_p, Wl_p, blin_eff = _host_prep(**inputs)
    in_maps = []
    for c in range(NCORES):
        blob = np.empty(TOTAL, np.float32)
        for j in range(BL):
            b = c * BL + j
            blob[XB + j * GW : XB + (j + 1) * GW] = xT[b].ravel()
            blob.view(np.uint8)[
                (AB + j * AWW) * 4 : (AB + (j + 1) * AWW) * 4
            ] = allowT[b].ravel()
        blob[MB : MB + 128 * H * DIN] = Mh.ravel()
        blob[VB : VB + 128 * H * DO] = Wv_p.ravel()
        blob[LB : LB + 128 * 4 * DLIN] = Wl_p.ravel()
        blob[BB : BB + 128] = blin_eff.astype(np.float32).ravel()
        in_maps.append({"blob": blob})
    return in_maps


def kernel(**inputs):
    from concourse import bass_utils

    nc = _get_nc()
    in_maps = make_in_maps(**inputs)
    res = bass_utils.run_bass_kernel_spmd(nc, in_maps, core_ids=list(range(NCORES)))
    y = np.concatenate([res.results[c]["y"] for c in range(NCORES)], axis=0)
    return np.ascontiguousarray(y.astype(np.float32))

